# revision 1
# baseline (speedup 1.0000x reference)
"""Trainium2 Bass kernel for nn_CrossAttention (8-core data-parallel over batch).

Reference math (per batch b, chunk c):
  en = LayerNorm(e) ; q = en@Wq+bq ; k = h@Wk+bk ; v = h@Wv+bv
  attn = softmax(q@k^T / 8) ; o = attn@v ; out = o@Wo + bo + e

Host-side folding:  q = ((e-mu)*rstd) @ (ln_g[:,None]*Wq/8) + (ln_b@Wq+bq)/8
so the on-chip LN is just (e-mu)*rstd.

On-chip dataflow (all matmuls fp32r):
  - activations transposed to d-major ("T") layout via PE transposes
  - qT projection with weights as stationary lhsT; biases added by K=1
    piggyback matmuls (ones-row x bias-row) inside each accumulation group
  - kT and v stored in block-diagonal head-pair layout so scores and AV are
    single full-array K=128 matmuls per pair (fp32r cannot write PSUM at a
    column offset, which rules out tile_position pairing)
  - softmax denominators via one accumulated E2 matmul -> psum [12, 256];
    1/den partition-broadcast via K=12 matmul with R_p selector constants;
    normalization applied at AV-psum evacuation (oT = pav * bcast)
  - O-proj, PE transpose back to row-major with residual fused into the
    evacuation, store from the e tile (never fp32r-consumed)
  - two-stage software pipeline: stage A (load/LN/xT/Q/vbd) of chunk c is
    emitted before stage B (attention/O/store) of chunk c-1
"""

import numpy as np

B, C, N, S, D = 8, 32, 4, 64, 768
NH, DK = 12, 64
R = N * S          # 256 rows per chunk
KO = D // 128      # 6 partition blocks of d
NP = 6             # head pairs
LN_EPS = 1e-5
GROUP = 4          # chunks per h/kv batch group

_prog_cache = {}


def _build(n_chunks):
    import concourse.bass as bass
    import concourse.tile as tile
    from concourse import bacc, mybir
    from contextlib import ExitStack

    F32 = mybir.dt.float32
    F32R = mybir.dt.float32r
    AF = mybir.ActivationFunctionType
    ALU = mybir.AluOpType

    nc = bacc.Bacc()

    d_e = nc.dram_tensor("e", [n_chunks, R, D], F32, kind="ExternalInput")
    d_h = nc.dram_tensor("h", [n_chunks, S, D], F32R, kind="ExternalInput")
    d_wq = nc.dram_tensor("wq", [KO, 128, D], F32R, kind="ExternalInput")
    d_wk = nc.dram_tensor("wk", [KO, 128, D], F32R, kind="ExternalInput")
    d_wv = nc.dram_tensor("wv", [KO, 128, D], F32R, kind="ExternalInput")
    d_wo = nc.dram_tensor("wo", [KO, 128, D], F32R, kind="ExternalInput")
    d_bqc = nc.dram_tensor("bqc", [128, KO], F32, kind="ExternalInput")
    d_bkc = nc.dram_tensor("bkc", [128, KO], F32, kind="ExternalInput")
    d_boc = nc.dram_tensor("boc", [128, KO], F32, kind="ExternalInput")
    d_e2c = nc.dram_tensor("e2c", [128, NP, NH], F32R, kind="ExternalInput")
    d_rpc = nc.dram_tensor("rpc", [NH, NP, 128], F32R, kind="ExternalInput")
    d_id = nc.dram_tensor("ident", [128, 128], F32R, kind="ExternalInput")
    d_zf = nc.dram_tensor("zf", [128, 1], F32R, kind="ExternalInput")
    d_ones = nc.dram_tensor("ones", [1, 512], F32R, kind="ExternalInput")
    d_bvrr = nc.dram_tensor("bvrr", [1, D], F32R, kind="ExternalInput")
    d_out = nc.dram_tensor("out", [n_chunks, R, D], F32, kind="ExternalOutput")

    with ExitStack() as ctx:
        tc = ctx.enter_context(tile.TileContext(nc))
        consts = ctx.enter_context(tc.tile_pool(name="consts", bufs=1))
        e_pool = ctx.enter_context(tc.tile_pool(name="e_pool", bufs=2))
        x_pool = ctx.enter_context(tc.tile_pool(name="x_pool", bufs=2))
        xT_pool = ctx.enter_context(tc.tile_pool(name="xT_pool", bufs=2))
        q_pool = ctx.enter_context(tc.tile_pool(name="q_pool", bufs=2))
        exp_pool = ctx.enter_context(tc.tile_pool(name="exp_pool", bufs=2))
        oT_pool = ctx.enter_context(tc.tile_pool(name="oT_pool", bufs=1))
        fT_pool = ctx.enter_context(tc.tile_pool(name="fT_pool", bufs=1))
        vd_pool = ctx.enter_context(tc.tile_pool(name="vd_pool", bufs=2))
        grp_pool = ctx.enter_context(tc.tile_pool(name="grp_pool", bufs=1))
        kt_pool = ctx.enter_context(tc.tile_pool(name="kt_pool", bufs=1))
        v2_pool = ctx.enter_context(tc.tile_pool(name="v2_pool", bufs=2))
        st_pool = ctx.enter_context(tc.tile_pool(name="st_pool", bufs=2))
        ps_mmA = ctx.enter_context(tc.tile_pool(name="ps_mmA", bufs=2, space="PSUM"))
        ps_mmB = ctx.enter_context(tc.tile_pool(name="ps_mmB", bufs=2, space="PSUM"))
        ps_s = ctx.enter_context(tc.tile_pool(name="ps_s", bufs=1, space="PSUM"))
        ps_ab = ctx.enter_context(tc.tile_pool(name="ps_ab", bufs=3, space="PSUM"))

        # ---- constants ----
        wq = consts.tile([128, KO, D], F32R)
        wk = consts.tile([128, KO, D], F32R)
        wv = consts.tile([128, KO, D], F32R)
        wo = consts.tile([128, KO, D], F32R)
        nc.sync.dma_start(wq[:], d_wq[:].rearrange("k p d -> p k d"))
        nc.sync.dma_start(wk[:], d_wk[:].rearrange("k p d -> p k d"))
        nc.sync.dma_start(wv[:], d_wv[:].rearrange("k p d -> p k d"))
        nc.sync.dma_start(wo[:], d_wo[:].rearrange("k p d -> p k d"))
        bqc = consts.tile([128, KO], F32)
        bkc = consts.tile([128, KO], F32)
        boc = consts.tile([128, KO], F32)
        nc.sync.dma_start(bqc[:], d_bqc[:])
        nc.sync.dma_start(bkc[:], d_bkc[:])
        nc.sync.dma_start(boc[:], d_boc[:])
        e2c = consts.tile([128, NP, NH], F32R)
        rpc = consts.tile([NH, NP, 128], F32R)
        ident = consts.tile([128, 128], F32R)
        zf = consts.tile([128, 1], F32R)
        ones = consts.tile([1, 512], F32R)
        bvrr = consts.tile([1, D], F32R)
        nc.sync.dma_start(e2c[:], d_e2c[:])
        nc.sync.dma_start(rpc[:], d_rpc[:])
        nc.sync.dma_start(ident[:], d_id[:])
        nc.sync.dma_start(zf[:], d_zf[:])
        nc.sync.dma_start(ones[:], d_ones[:])
        nc.sync.dma_start(bvrr[:], d_bvrr[:])
        eps_t = consts.tile([128, 1], F32)
        nc.vector.memset(eps_t[:], LN_EPS)

        def group_phase(g):
            # h load (halves), transpose, K/V projections
            hT4 = grp_pool.tile([128, KO, GROUP * S], F32R, tag="hT4")
            for hh in range(2):
                h2 = grp_pool.tile([S, 2, D], F32R, tag="h2")
                nc.sync.dma_start(
                    h2[:], d_h[g * GROUP + 2 * hh:g * GROUP + 2 * hh + 2]
                    .rearrange("c j d -> j c d"))
                for c2 in range(2):
                    cc = 2 * hh + c2
                    for k0, kn in ((0, 4), (4, 2)):
                        ptq = ps_mmA.tile([128, 4, 128], F32R, tag="mmA")
                        for i in range(kn):
                            nc.tensor.transpose(
                                ptq[:, i, 0:S],
                                h2[:, c2, (k0 + i) * 128:(k0 + i + 1) * 128],
                                ident[0:S, 0:S])
                        nc.vector.tensor_copy(
                            hT4[:, k0:k0 + kn, cc * S:(cc + 1) * S],
                            ptq[:, 0:kn, 0:S])

            # kT in block-diagonal pair layout
            kbd = kt_pool.tile([128, NP, GROUP, 128], F32R, tag="kbd")
            nc.gpsimd.tensor_copy(
                kbd[:], zf[:, None, None, 0:1].to_broadcast(
                    [128, NP, GROUP, 128]))
            for mo in range(KO):
                pk = ps_mmA.tile([128, 512], F32, tag="mmA")
                for ko in range(KO):
                    nc.tensor.matmul(
                        pk[:, 0:GROUP * S],
                        wk[:, ko, mo * 128:(mo + 1) * 128],
                        hT4[:, ko, :],
                        start=(ko == 0), stop=(ko == KO - 1))
                pkv = pk[:, 0:GROUP * S].rearrange("p (c j) -> p c j", c=GROUP)
                nc.vector.tensor_scalar(
                    out=kbd[0:64, mo, :, 0:S], in0=pkv[0:64],
                    scalar1=bkc[0:64, mo:mo + 1], scalar2=None, op0=ALU.add)
                nc.vector.tensor_scalar(
                    out=kbd[64:128, mo, :, S:128], in0=pkv[64:128],
                    scalar1=bkc[64:128, mo:mo + 1], scalar2=None, op0=ALU.add)

            v2 = []
            for st in range(GROUP // 2):
                v2t = v2_pool.tile([128, D], F32R, tag="v2")
                for n0, ns in ((0, 512), (512, 256)):
                    pv = ps_mmA.tile([128, 512], F32, tag="mmA")
                    for ko in range(KO):
                        nc.tensor.matmul(
                            pv[:, 0:ns],
                            hT4[:, ko, st * 128:(st + 1) * 128],
                            wv[:, ko, n0:n0 + ns],
                            start=(ko == 0), stop=False)
                    nc.tensor.matmul(
                        pv[:, 0:ns], ones[:, 0:128],
                        bvrr[:, n0:n0 + ns], start=False, stop=True)
                    nc.vector.tensor_copy(v2t[:, n0:n0 + ns], pv[:, 0:ns])
                v2.append(v2t)
            return kbd, v2

        def stage_a(c, hctx):
            kbd, v2 = hctx
            cc = c % GROUP
            # ---- load e, LayerNorm stats + apply ----
            e_sb = e_pool.tile([128, 2, D], F32, tag="e")
            nc.sync.dma_start(
                e_sb[:], d_e[c].rearrange("(t p) d -> p t d", p=128))

            stats = st_pool.tile([128, 2, 3, 6], F32, tag="stats")
            mv = st_pool.tile([128, 2, 2], F32, tag="mv")
            rstd = st_pool.tile([128, 2], F32, tag="rstd")
            x_sb = x_pool.tile([128, 2, D], F32R, tag="x")
            for t in range(2):
                esl = e_sb[:, t, :].rearrange("p (s f) -> p s f", s=3)
                for sg in range(3):
                    nc.vector.bn_stats(stats[:, t, sg, :], esl[:, sg, :])
                nc.vector.bn_aggr(mv[:, t, :], stats[:, t, :, :])
            # rstd = rsqrt(var + eps) via bit-hack + 2 Newton steps (DVE only,
            # keeps Sqrt out of ACT so no act-table reloads)
            I32 = mybir.dt.int32
            v1 = st_pool.tile([128, 2], F32, tag="v1")
            y = st_pool.tile([128, 2], F32, tag="y")
            tmp = st_pool.tile([128, 2], F32, tag="tmp")
            nc.vector.tensor_scalar(
                out=v1[:], in0=mv[:, :, 1], scalar1=float(LN_EPS), scalar2=None,
                op0=ALU.add)
            nc.vector.tensor_scalar(
                out=y[:].bitcast(I32), in0=v1[:].bitcast(I32), scalar1=1,
                scalar2=None, op0=ALU.logical_shift_right)
            nc.vector.tensor_scalar(
                out=y[:].bitcast(I32), in0=y[:].bitcast(I32), scalar1=-1,
                scalar2=0x5F3759DF, op0=ALU.mult, op1=ALU.add)
            for _ in range(2):
                nc.vector.tensor_tensor(
                    out=tmp[:], in0=y[:], in1=y[:], op=ALU.mult)
                nc.vector.tensor_tensor(
                    out=tmp[:], in0=tmp[:], in1=v1[:], op=ALU.mult)
                nc.vector.tensor_scalar(
                    out=tmp[:], in0=tmp[:], scalar1=-0.5, scalar2=1.5,
                    op0=ALU.mult, op1=ALU.add)
                nc.vector.tensor_tensor(
                    out=rstd[:], in0=y[:], in1=tmp[:], op=ALU.mult)
                nc.vector.tensor_copy(y[:], rstd[:])
            for t in range(2):
                nc.gpsimd.tensor_scalar(
                    out=x_sb[:, t, :], in0=e_sb[:, t, :],
                    scalar1=mv[:, t, 0:1], scalar2=rstd[:, t:t + 1],
                    op0=ALU.subtract, op1=ALU.mult)

            # ---- transpose x to d-major ----
            xT = xT_pool.tile([128, KO, R], F32R, tag="xT")
            for ko2 in range(KO // 2):
                pt4 = ps_mmA.tile([128, 4, 128], F32R, tag="mmA")
                for i in range(2):
                    for t in range(2):
                        nc.tensor.transpose(
                            pt4[:, 2 * i + t, :],
                            x_sb[:, t, (2 * ko2 + i) * 128:(2 * ko2 + i + 1) * 128],
                            ident[:])
                nc.scalar.copy(xT[:, 2 * ko2:2 * ko2 + 2, :], pt4[:])

            # ---- Q projection ----
            qT = q_pool.tile([128, KO, R], F32R, tag="qT")
            for mo in range(KO):
                pq = ps_mmA.tile([128, 512], F32, tag="mmA")
                for ko in range(KO):
                    nc.tensor.matmul(
                        pq[:, 0:R], wq[:, ko, mo * 128:(mo + 1) * 128],
                        xT[:, ko, :], start=(ko == 0), stop=(ko == KO - 1))
                nc.scalar.activation(
                    qT[:, mo, :], pq[:, 0:R], AF.Identity,
                    bias=bqc[:, mo:mo + 1], scale=1.0)

            # ---- v in block-diagonal pair layout ----
            v2t = v2[cc // 2]
            pa = 64 * (cc % 2)
            vbd = vd_pool.tile([128, NP, 128], F32R, tag="vbd")
            nc.gpsimd.tensor_copy(
                vbd[:], zf[:, None, 0:1].to_broadcast([128, NP, 128]))
            v2v = v2t[pa:pa + 64, :].rearrange(
                "p (np two dk) -> p np two dk", np=NP, two=2)
            nc.gpsimd.tensor_copy(vbd[0:64, :, 0:DK], v2v[:, :, 0, :])
            nc.gpsimd.tensor_copy(vbd[64:128, :, DK:128], v2v[:, :, 1, :])
            return (c, e_sb, x_sb, xT, qT, vbd)

        def stage_b(actx, hctx):
            c, e_sb, x_sb, xT, qT, vbd = actx
            kbd, v2 = hctx
            cc = c % GROUP

            # ---- attention (head-pairs processed two at a time) ----
            expT = exp_pool.tile([128, NP, R], F32R, tag="expT")
            pden_t = ps_s.tile([128, 2, R], F32, tag="s", name="pden_t")
            pden = pden_t[0:NH, 0, :]
            for p2 in range(0, NP, 2):
                pscr = ps_s.tile([128, 2, R], F32, tag="s")
                for i in range(2):
                    nc.tensor.matmul(
                        pscr[:, i, :], kbd[:, p2 + i, cc, :], qT[:, p2 + i, :],
                        start=True, stop=True)
                nc.scalar.activation(
                    expT[:, p2:p2 + 2, :], pscr[:], AF.Exp, bias=0.0, scale=1.0)
            for p in range(NP):
                nc.tensor.matmul(
                    pden[:], e2c[:, p, :], expT[:, p, :],
                    start=(p == 0), stop=(p == NP - 1),
                    skip_group_check=True)

            recip = st_pool.tile([NH, R], F32R, tag="recip")
            with nc.allow_low_precision(reason="fp32r softmax denom"):
                nc.vector.reciprocal(recip[:], pden[:])

            oT = oT_pool.tile([128, KO, R], F32R, tag="oT")
            for p2 in range(0, NP, 2):
                pav = ps_ab.tile([128, 2, R], F32, tag="ab")
                pbc = ps_ab.tile([128, 2, R], F32, tag="ab")
                for i in range(2):
                    nc.tensor.matmul(
                        pav[:, i, :], vbd[:, p2 + i, :], expT[:, p2 + i, :],
                        start=True, stop=True)
                    nc.tensor.matmul(
                        pbc[:, i, :], rpc[:, p2 + i, :], recip[:],
                        start=True, stop=True)
                bc_sb = st_pool.tile([128, 2, R], F32, tag="bcsb")
                nc.scalar.copy(bc_sb[:], pbc[:])
                nc.vector.tensor_tensor(
                    out=oT[:, p2:p2 + 2, :], in0=pav[:], in1=bc_sb[:],
                    op=ALU.mult)

            # ---- O projection ----
            fT = fT_pool.tile([128, KO, R], F32R, tag="fT")
            for mo in range(KO):
                pf = ps_mmB.tile([128, 512], F32, tag="mmB")
                for ko in range(KO):
                    nc.tensor.matmul(
                        pf[:, 0:R], wo[:, ko, mo * 128:(mo + 1) * 128],
                        oT[:, ko, :], start=(ko == 0), stop=(ko == KO - 1))
                nc.scalar.activation(
                    fT[:, mo, :], pf[:, 0:R], AF.Identity,
                    bias=boc[:, mo:mo + 1], scale=1.0)

            # ---- transpose back + residual, store ----
            for t in range(2):
                for m0, mn in ((0, 4), (4, 2)):
                    ptq = ps_mmB.tile([128, 4, 128], F32R, tag="mmB")
                    for i in range(mn):
                        nc.tensor.transpose(
                            ptq[:, i, :], fT[:, m0 + i, t * 128:(t + 1) * 128],
                            ident[:])
                    nc.vector.tensor_tensor(
                        out=e_sb[:, t, m0 * 128:(m0 + mn) * 128],
                        in0=ptq[:, 0:mn, :].bitcast(F32),
                        in1=e_sb[:, t, m0 * 128:(m0 + mn) * 128],
                        op=ALU.add)
            nc.sync.dma_start(
                d_out[c].rearrange("(t p) d -> p t d", p=128), e_sb[:])

        # ---- software-pipelined driver: A(c+1) emitted ahead of B(c) ----
        n_groups = n_chunks // GROUP
        pending = None
        for g in range(n_groups):
            hctx = group_phase(g)
            for cc in range(GROUP):
                actx = stage_a(g * GROUP + cc, hctx)
                if pending is not None:
                    stage_b(*pending)
                pending = (actx, hctx)
        if pending is not None:
            stage_b(*pending)

    nc.compile()
    return nc


def _prep_consts(Wq, bq, Wk, bk, Wv, bv, Wo, bo, ln_g, ln_b):
    scale = 1.0 / np.sqrt(DK)
    Wq_eff = (ln_g[:, None] * Wq) * scale
    bq_eff = (ln_b @ Wq + bq) * scale

    def wl(w):
        return np.ascontiguousarray(w.reshape(KO, 128, D), dtype=np.float32)

    e2c = np.zeros((128, NP, NH), dtype=np.float32)
    for p in range(NP):
        e2c[0:64, p, 2 * p] = 1.0
        e2c[64:128, p, 2 * p + 1] = 1.0
    rpc = np.zeros((NH, NP, 128), dtype=np.float32)
    for p in range(NP):
        rpc[2 * p, p, 0:64] = 1.0
        rpc[2 * p + 1, p, 64:128] = 1.0

    return {
        "wq": wl(Wq_eff), "wk": wl(Wk), "wv": wl(Wv), "wo": wl(Wo),
        "e2c": e2c, "rpc": rpc, "zf": np.zeros((128, 1), dtype=np.float32),
        "ones": np.ones((1, 512), dtype=np.float32),
        "bqc": np.ascontiguousarray(bq_eff.reshape(KO, 128).T, dtype=np.float32),
        "bkc": np.ascontiguousarray(bk.reshape(KO, 128).T, dtype=np.float32),
        "boc": np.ascontiguousarray(bo.reshape(KO, 128).T, dtype=np.float32),
        "bvrr": np.ascontiguousarray(bv.reshape(1, D), dtype=np.float32),
        "ident": np.eye(128, dtype=np.float32),
    }


def kernel(e, h, Wq, bq, Wk, bk, Wv, bv, Wo, bo, ln_g, ln_b):
    from concourse.bass_utils import run_bass_kernel_spmd

    e = np.asarray(e, dtype=np.float32)
    h = np.asarray(h, dtype=np.float32)
    n_chunks = e.shape[1]

    if n_chunks not in _prog_cache:
        _prog_cache[n_chunks] = _build(n_chunks)
    nc = _prog_cache[n_chunks]

    consts = _prep_consts(
        np.asarray(Wq, np.float32), np.asarray(bq, np.float32),
        np.asarray(Wk, np.float32), np.asarray(bk, np.float32),
        np.asarray(Wv, np.float32), np.asarray(bv, np.float32),
        np.asarray(Wo, np.float32), np.asarray(bo, np.float32),
        np.asarray(ln_g, np.float32), np.asarray(ln_b, np.float32))

    in_maps = []
    for b in range(B):
        m = dict(consts)
        m["e"] = np.ascontiguousarray(e[b].reshape(n_chunks, R, D))
        m["h"] = np.ascontiguousarray(h[b])
        in_maps.append(m)

    res = run_bass_kernel_spmd(nc, in_maps, core_ids=list(range(B)))
    out = np.stack([r["out"] for r in res.results], axis=0)
    return out.reshape(B, n_chunks, N, S, D)



# revision 3
# speedup vs baseline: 1.3095x; 1.3095x over previous
"""Trainium2 Bass kernel for nn_CrossAttention (8-core data-parallel over batch).

Reference math (per batch b, chunk c):
  en = LayerNorm(e) ; q = en@Wq+bq ; k = h@Wk+bk ; v = h@Wv+bv
  attn = softmax(q@k^T / 8) ; o = attn@v ; out = o@Wo + bo + e

v2 design (vs fp32r baseline):
  - all four 768x768 projections run as fp8(e4m3) DoubleRow matmuls
    (0.5 cyc/row): weights are host-scaled x8 (to keep fp8 mantissa) and
    the 1/8 is folded back at PSUM evacuation
  - attention core (scores / den / AV / bcast) stays bf16 for accuracy
  - O-projection is computed ROW-major (lhsT = oT fp8), so there is no
    transpose-back; the residual (e + bo, folded host-side) is added by the
    PSUM-evacuating scalar_tensor_tensor
  - block-diagonal kbd/vbd tiles are persistent and zero-filled ONCE
    (the baseline re-broadcast zeros every chunk on the Pool engine)
  - LayerNorm: bn_stats + quake-rsqrt (1 Newton step) on DVE, normalize on
    ACT (scale=rstd, bias=-mu*rstd per-partition APs) writing fp8 directly
  - elementwise work spread across ACT (normalize, Q-evac, exp, xT-evac)
    DVE (stats, oT=pav*pbc, residual) and Pool (kv-side evacs, vbd build)
"""

import numpy as np

B, C, N, S, D = 8, 32, 4, 64, 768
NH, DK = 12, 64
R = N * S          # 256 rows per chunk
KO = D // 128      # 6 partition blocks of d
NP = 6             # head pairs
LN_EPS = 1e-5
GROUP = 4          # chunks per h/kv batch group

_prog_cache = {}


def _build(n_chunks):
    import concourse.bass as bass
    import concourse.tile as tile
    from concourse import bacc, mybir
    from contextlib import ExitStack

    F32 = mybir.dt.float32
    F32R = mybir.dt.float32r
    BF16 = mybir.dt.bfloat16
    FP8 = mybir.dt.float8e4
    I32 = mybir.dt.int32
    AF = mybir.ActivationFunctionType
    ALU = mybir.AluOpType
    DR = mybir.MatmulPerfMode.DoubleRow

    nc = bacc.Bacc()

    d_e = nc.dram_tensor("e", [n_chunks, R, D], F32, kind="ExternalInput")
    d_h = nc.dram_tensor("h", [n_chunks, S, D], F32R, kind="ExternalInput")
    d_wq = nc.dram_tensor("wq", [KO, 128, D], FP8, kind="ExternalInput")
    d_wk = nc.dram_tensor("wk", [KO, 128, D], FP8, kind="ExternalInput")
    d_wv = nc.dram_tensor("wv", [KO, 128, D], FP8, kind="ExternalInput")
    d_wo = nc.dram_tensor("wo", [KO, 128, D], FP8, kind="ExternalInput")
    d_bqc = nc.dram_tensor("bqc", [128, KO], F32, kind="ExternalInput")
    d_bkc = nc.dram_tensor("bkc", [128, KO], F32, kind="ExternalInput")
    d_e2c = nc.dram_tensor("e2c", [128, NP, NH], BF16, kind="ExternalInput")
    d_rpc = nc.dram_tensor("rpc", [NH, NP, 128], BF16, kind="ExternalInput")
    d_idr = nc.dram_tensor("identr", [128, 128], F32R, kind="ExternalInput")
    d_id8 = nc.dram_tensor("ident8", [128, 128], FP8, kind="ExternalInput")
    d_ones = nc.dram_tensor("onesb", [1, 128], BF16, kind="ExternalInput")
    d_bvrr = nc.dram_tensor("bvrr", [1, D], BF16, kind="ExternalInput")
    d_out = nc.dram_tensor("out", [n_chunks, R, D], F32, kind="ExternalOutput")

    with ExitStack() as ctx:
        tc = ctx.enter_context(tile.TileContext(nc))
        consts = ctx.enter_context(tc.tile_pool(name="consts", bufs=1))
        e_pool = ctx.enter_context(tc.tile_pool(name="e_pool", bufs=2))
        x_pool = ctx.enter_context(tc.tile_pool(name="x_pool", bufs=2))
        xT_pool = ctx.enter_context(tc.tile_pool(name="xT_pool", bufs=2))
        q_pool = ctx.enter_context(tc.tile_pool(name="q_pool", bufs=2))
        exp_pool = ctx.enter_context(tc.tile_pool(name="exp_pool", bufs=2))
        oT_pool = ctx.enter_context(tc.tile_pool(name="oT_pool", bufs=2))
        st_pool = ctx.enter_context(tc.tile_pool(name="st_pool", bufs=2))
        grp_pool = ctx.enter_context(tc.tile_pool(name="grp_pool", bufs=1))
        v2_pool = ctx.enter_context(tc.tile_pool(name="v2_pool", bufs=2))
        ps_t = ctx.enter_context(tc.tile_pool(name="ps_t", bufs=1, space="PSUM"))
        ps_q = ctx.enter_context(tc.tile_pool(name="ps_q", bufs=1, space="PSUM"))
        ps_s = ctx.enter_context(tc.tile_pool(name="ps_s", bufs=2, space="PSUM"))
        ps_av = ctx.enter_context(tc.tile_pool(name="ps_av", bufs=1, space="PSUM"))
        ps_bc = ctx.enter_context(tc.tile_pool(name="ps_bc", bufs=1, space="PSUM"))
        ps_o5 = ctx.enter_context(tc.tile_pool(name="ps_o5", bufs=1, space="PSUM"))
        ps_o2 = ctx.enter_context(tc.tile_pool(name="ps_o2", bufs=1, space="PSUM"))

        # ---- constants ----
        wq8 = consts.tile([128, KO, D], FP8)
        wk8 = consts.tile([128, KO, D], FP8)
        wv8 = consts.tile([128, KO, D], FP8)
        wo8 = consts.tile([128, KO, D], FP8)
        nc.sync.dma_start(wq8[:], d_wq[:].rearrange("k p d -> p k d"))
        nc.sync.dma_start(wk8[:], d_wk[:].rearrange("k p d -> p k d"))
        nc.sync.dma_start(wv8[:], d_wv[:].rearrange("k p d -> p k d"))
        nc.sync.dma_start(wo8[:], d_wo[:].rearrange("k p d -> p k d"))
        bqc = consts.tile([128, KO], F32)
        bkc = consts.tile([128, KO], F32)
        nc.sync.dma_start(bqc[:], d_bqc[:])
        nc.sync.dma_start(bkc[:], d_bkc[:])
        e2c = consts.tile([128, NP, NH], BF16)
        rpc = consts.tile([NH, NP, 128], BF16)
        identr = consts.tile([128, 128], F32R)
        ident8 = consts.tile([128, 128], FP8)
        onesb = consts.tile([1, 128], BF16)
        bvrr = consts.tile([1, D], BF16)
        nc.sync.dma_start(e2c[:], d_e2c[:])
        nc.sync.dma_start(rpc[:], d_rpc[:])
        nc.sync.dma_start(identr[:], d_idr[:])
        nc.sync.dma_start(ident8[:], d_id8[:])
        nc.sync.dma_start(onesb[:], d_ones[:])
        nc.sync.dma_start(bvrr[:], d_bvrr[:])

        # persistent block-diagonal tiles: zero-fill ONCE, only diagonal
        # blocks are rewritten (off-diag stays zero forever)
        kbd2 = [consts.tile([128, NP, GROUP, 128], BF16, name=f"kbd{i}")
                for i in range(2)]
        vbd2 = [consts.tile([128, NP, 128], BF16, name=f"vbd{i}")
                for i in range(2)]
        for t_ in kbd2 + vbd2:
            nc.gpsimd.memset(t_[:], 0.0)

        def group_phase(g):
            # h load (halves), transpose (fp32r), K/V projections (fp8 DR)
            hT4 = grp_pool.tile([128, KO, GROUP * S], FP8, tag="hT4")
            for hh in range(2):
                h2 = grp_pool.tile([S, 2, D], F32R, tag="h2")
                nc.sync.dma_start(
                    h2[:], d_h[g * GROUP + 2 * hh:g * GROUP + 2 * hh + 2]
                    .rearrange("c j d -> j c d"))
                for c2 in range(2):
                    cc = 2 * hh + c2
                    pth = ps_t.tile([128, KO, S], F32R, tag="t8", name="pth")
                    for i in range(KO):
                        nc.tensor.transpose(
                            pth[:, i, :],
                            h2[:, c2, i * 128:(i + 1) * 128],
                            identr[0:S, 0:S])
                    nc.gpsimd.tensor_copy(
                        hT4[:, :, cc * S:(cc + 1) * S], pth[:])

            # kT in block-diagonal pair layout (persistent tile g%2)
            kbd = kbd2[g % 2]
            pk = ps_q.tile([128, 2, 256], F32, tag="q", name="pk")
            for mo in range(KO):
                pkh = pk[:, mo % 2, :]
                for k2 in range(3):
                    nc.tensor.matmul(
                        pkh,
                        wk8[:, 2 * k2:2 * k2 + 2, mo * 128:(mo + 1) * 128],
                        hT4[:, 2 * k2:2 * k2 + 2, :],
                        start=(k2 == 0), stop=(k2 == 2), perf_mode=DR)
                pkv = pkh.rearrange("p (c j) -> p c j", c=GROUP)
                nc.gpsimd.tensor_scalar(
                    out=kbd[0:64, mo, :, 0:S], in0=pkv[0:64],
                    scalar1=0.125, scalar2=bkc[0:64, mo:mo + 1],
                    op0=ALU.mult, op1=ALU.add)
                nc.gpsimd.tensor_scalar(
                    out=kbd[64:128, mo, :, S:128], in0=pkv[64:128],
                    scalar1=0.125, scalar2=bkc[64:128, mo:mo + 1],
                    op0=ALU.mult, op1=ALU.add)

            v2 = []
            for st in range(GROUP // 2):
                v2t = v2_pool.tile([128, D], BF16, tag="v2")
                pv5 = ps_o5.tile([128, 512], F32, tag="o5", name="pv5")
                pv2 = ps_o2.tile([128, 256], F32, tag="o2", name="pv2")
                for k2 in range(3):
                    nc.tensor.matmul(
                        pv5[:],
                        hT4[:, 2 * k2:2 * k2 + 2, st * 128:(st + 1) * 128],
                        wv8[:, 2 * k2:2 * k2 + 2, 0:512],
                        start=(k2 == 0), stop=False, perf_mode=DR)
                    nc.tensor.matmul(
                        pv2[:],
                        hT4[:, 2 * k2:2 * k2 + 2, st * 128:(st + 1) * 128],
                        wv8[:, 2 * k2:2 * k2 + 2, 512:768],
                        start=(k2 == 0), stop=False, perf_mode=DR)
                nc.tensor.matmul(
                    pv5[:], onesb[:], bvrr[:, 0:512], start=False, stop=True)
                nc.tensor.matmul(
                    pv2[:], onesb[:], bvrr[:, 512:768], start=False, stop=True)
                nc.vector.tensor_scalar(
                    out=v2t[:, 0:512], in0=pv5[:], scalar1=0.125,
                    scalar2=None, op0=ALU.mult)
                nc.vector.tensor_scalar(
                    out=v2t[:, 512:768], in0=pv2[:], scalar1=0.125,
                    scalar2=None, op0=ALU.mult)
                v2.append(v2t)
            return kbd, v2

        def stage_a(c, hctx):
            kbd, v2 = hctx
            cc = c % GROUP
            # ---- load e (residual is pre-folded with bo host-side) ----
            e_sb = e_pool.tile([128, 2, D], F32, tag="e")
            nc.sync.dma_start(
                e_sb[:], d_e[c].rearrange("(t p) d -> p t d", p=128))

            # ---- LN stats on DVE ----
            stats = st_pool.tile([128, 2, 3, 6], F32, tag="stats")
            mv = st_pool.tile([128, 2, 2], F32, tag="mv")
            rstd = st_pool.tile([128, 2], F32, tag="rstd")
            nmr = st_pool.tile([128, 2], F32, tag="nmr")
            for t in range(2):
                esl = e_sb[:, t, :].rearrange("p (s f) -> p s f", s=3)
                for sg in range(3):
                    nc.vector.bn_stats(stats[:, t, sg, :], esl[:, sg, :])
                nc.vector.bn_aggr(mv[:, t, :], stats[:, t, :, :])
            # rstd = rsqrt(var+eps): quake bit-hack + 1 Newton step (DVE)
            v1 = st_pool.tile([128, 2], F32, tag="v1")
            y = st_pool.tile([128, 2], F32, tag="y")
            tmp = st_pool.tile([128, 2], F32, tag="tmp")
            nc.vector.tensor_scalar(
                out=v1[:], in0=mv[:, :, 1], scalar1=float(LN_EPS), scalar2=None,
                op0=ALU.add)
            nc.vector.tensor_scalar(
                out=y[:].bitcast(I32), in0=v1[:].bitcast(I32), scalar1=1,
                scalar2=None, op0=ALU.logical_shift_right)
            nc.vector.tensor_scalar(
                out=y[:].bitcast(I32), in0=y[:].bitcast(I32), scalar1=-1,
                scalar2=0x5F3759DF, op0=ALU.mult, op1=ALU.add)
            nc.vector.tensor_tensor(out=tmp[:], in0=y[:], in1=y[:], op=ALU.mult)
            nc.vector.tensor_tensor(out=tmp[:], in0=tmp[:], in1=v1[:], op=ALU.mult)
            nc.vector.tensor_scalar(
                out=tmp[:], in0=tmp[:], scalar1=-0.5, scalar2=1.5,
                op0=ALU.mult, op1=ALU.add)
            nc.vector.tensor_tensor(out=rstd[:], in0=y[:], in1=tmp[:], op=ALU.mult)
            nc.vector.scalar_tensor_tensor(
                out=nmr[:], in0=mv[:, :, 0], scalar=-1.0, in1=rstd[:],
                op0=ALU.mult, op1=ALU.mult)

            # ---- normalize on ACT -> x8 (fp8) ----
            x8 = x_pool.tile([128, 2, D], FP8, tag="x")
            for t in range(2):
                nc.scalar.activation(
                    x8[:, t, :], e_sb[:, t, :], AF.Identity,
                    bias=nmr[:, t:t + 1], scale=rstd[:, t:t + 1])

            # ---- transpose x8 to d-major (fp8, PE), evac on ACT ----
            pt = ps_t.tile([128, 2, KO, 128], FP8, tag="t8", name="pt")
            for t in range(2):
                for i in range(KO):
                    nc.tensor.transpose(
                        pt[:, t, i, :], x8[:, t, i * 128:(i + 1) * 128],
                        ident8[:])
            xT8 = xT_pool.tile([128, KO, R], FP8, tag="xT")
            for t in range(2):
                nc.scalar.copy(
                    xT8[:, :, t * 128:(t + 1) * 128], pt[:, t, :, :])

            # ---- Q projection (fp8 DR), evac on ACT with 1/64 + bias ----
            qT = q_pool.tile([128, KO, R], BF16, tag="qT")
            pq = ps_q.tile([128, 2, 256], F32, tag="q", name="pq")
            for mo in range(KO):
                pqh = pq[:, mo % 2, :]
                for k2 in range(3):
                    nc.tensor.matmul(
                        pqh,
                        wq8[:, 2 * k2:2 * k2 + 2, mo * 128:(mo + 1) * 128],
                        xT8[:, 2 * k2:2 * k2 + 2, :],
                        start=(k2 == 0), stop=(k2 == 2), perf_mode=DR)
                nc.scalar.activation(
                    qT[:, mo, :], pqh, AF.Identity,
                    bias=bqc[:, mo:mo + 1], scale=0.015625)

            # ---- v in block-diagonal pair layout (persistent tile cc%2) ----
            v2t = v2[cc // 2]
            pa = 64 * (cc % 2)
            vbd = vbd2[cc % 2]
            v2v = v2t[pa:pa + 64, :].rearrange(
                "p (np two dk) -> p np two dk", np=NP, two=2)
            # the half whose partitions line up goes on DVE (bf16 fast mode);
            # the cross-partition half must go through gpsimd
            if cc % 2 == 0:
                nc.vector.tensor_copy(vbd[0:64, :, 0:DK], v2v[:, :, 0, :])
                nc.gpsimd.tensor_copy(vbd[64:128, :, DK:128], v2v[:, :, 1, :])
            else:
                nc.gpsimd.tensor_copy(vbd[0:64, :, 0:DK], v2v[:, :, 0, :])
                nc.vector.tensor_copy(vbd[64:128, :, DK:128], v2v[:, :, 1, :])
            return (c, e_sb, qT, vbd)

        def stage_b(actx, hctx):
            c, e_sb, qT, vbd = actx
            kbd, v2 = hctx
            cc = c % GROUP

            # ---- scores (bf16) + exp on ACT ----
            expT = exp_pool.tile([128, NP, R], BF16, tag="expT")
            for p2 in range(0, NP, 2):
                pscr = ps_s.tile([128, 2, R], F32, tag="s")
                for i in range(2):
                    nc.tensor.matmul(
                        pscr[:, i, :], kbd[:, p2 + i, cc, :], qT[:, p2 + i, :],
                        start=True, stop=True)
                nc.scalar.activation(
                    expT[:, p2:p2 + 2, :], pscr[:], AF.Exp, bias=0.0, scale=1.0)

            # ---- softmax denominators (bf16 matmul) + reciprocal ----
            pden_t = ps_s.tile([128, R], F32, tag="s", name="pden_t")
            pden = pden_t[0:NH, :]
            for p in range(NP):
                nc.tensor.matmul(
                    pden, e2c[:, p, :], expT[:, p, :],
                    start=(p == 0), stop=(p == NP - 1))
            recip = st_pool.tile([NH, R], BF16, tag="recip")
            with nc.allow_low_precision(reason="bf16 softmax denom"):
                nc.vector.reciprocal(recip[:], pden)

            # ---- AV + recip broadcast + normalize into oT (fp8) ----
            oT = oT_pool.tile([128, KO, R], FP8, tag="oT")
            for p2 in range(0, NP, 2):
                pav = ps_av.tile([128, 2, R], F32, tag="av")
                pbc = ps_bc.tile([128, 2, R], F32, tag="bc")
                for i in range(2):
                    nc.tensor.matmul(
                        pav[:, i, :], vbd[:, p2 + i, :], expT[:, p2 + i, :],
                        start=True, stop=True)
                    nc.tensor.matmul(
                        pbc[:, i, :], rpc[:, p2 + i, :], recip[:],
                        start=True, stop=True)
                with nc.allow_low_precision(reason="fp8 oT"):
                    nc.vector.tensor_tensor(
                        out=oT[:, p2:p2 + 2, :], in0=pav[:], in1=pbc[:],
                        op=ALU.mult)

            # ---- O projection ROW-major (fp8 DR) + residual evac ----
            for rb in range(2):
                po5 = ps_o5.tile([128, 512], F32, tag="o5", name="po5")
                po2 = ps_o2.tile([128, 256], F32, tag="o2", name="po2")
                for k2 in range(3):
                    nc.tensor.matmul(
                        po5[:],
                        oT[:, 2 * k2:2 * k2 + 2, rb * 128:(rb + 1) * 128],
                        wo8[:, 2 * k2:2 * k2 + 2, 0:512],
                        start=(k2 == 0), stop=(k2 == 2), perf_mode=DR)
                    nc.tensor.matmul(
                        po2[:],
                        oT[:, 2 * k2:2 * k2 + 2, rb * 128:(rb + 1) * 128],
                        wo8[:, 2 * k2:2 * k2 + 2, 512:768],
                        start=(k2 == 0), stop=(k2 == 2), perf_mode=DR)
                nc.vector.scalar_tensor_tensor(
                    out=e_sb[:, rb, 0:512], in0=po5[:], scalar=0.125,
                    in1=e_sb[:, rb, 0:512], op0=ALU.mult, op1=ALU.add)
                nc.gpsimd.scalar_tensor_tensor(
                    out=e_sb[:, rb, 512:768], in0=po2[:], scalar=0.125,
                    in1=e_sb[:, rb, 512:768], op0=ALU.mult, op1=ALU.add)
            nc.sync.dma_start(
                d_out[c].rearrange("(t p) d -> p t d", p=128), e_sb[:])

        # ---- software-pipelined driver: A(c+1) emitted ahead of B(c) ----
        n_groups = n_chunks // GROUP
        pending = None
        for g in range(n_groups):
            hctx = group_phase(g)
            for cc in range(GROUP):
                actx = stage_a(g * GROUP + cc, hctx)
                if pending is not None:
                    stage_b(*pending)
                pending = (actx, hctx)
        if pending is not None:
            stage_b(*pending)

    nc.compile()
    return nc


def _prep_consts(Wq, bq, Wk, bk, Wv, bv, Wo, bo, ln_g, ln_b):
    import ml_dtypes
    FP8 = ml_dtypes.float8_e4m3
    BF16 = ml_dtypes.bfloat16

    Wq_l = ln_g[:, None] * Wq
    bq_eff = (ln_b @ Wq + bq) * 0.125

    def wl8(w):
        return np.ascontiguousarray(
            (8.0 * w).reshape(KO, 128, D).astype(FP8))

    e2c = np.zeros((128, NP, NH), dtype=np.float32)
    for p in range(NP):
        e2c[0:64, p, 2 * p] = 1.0
        e2c[64:128, p, 2 * p + 1] = 1.0
    rpc = np.zeros((NH, NP, 128), dtype=np.float32)
    for p in range(NP):
        rpc[2 * p, p, 0:64] = 1.0
        rpc[2 * p + 1, p, 64:128] = 1.0

    return {
        "wq": wl8(Wq_l), "wk": wl8(Wk), "wv": wl8(Wv), "wo": wl8(Wo),
        "e2c": e2c.astype(BF16), "rpc": rpc.astype(BF16),
        "onesb": np.ones((1, 128), dtype=np.float32).astype(BF16),
        "bqc": np.ascontiguousarray(bq_eff.reshape(KO, 128).T, dtype=np.float32),
        "bkc": np.ascontiguousarray(bk.reshape(KO, 128).T, dtype=np.float32),
        "bvrr": np.ascontiguousarray((8.0 * bv).reshape(1, D)).astype(BF16),
        "identr": np.eye(128, dtype=np.float32),
        "ident8": np.eye(128, dtype=np.float32).astype(FP8),
    }


def kernel(e, h, Wq, bq, Wk, bk, Wv, bv, Wo, bo, ln_g, ln_b):
    from concourse.bass_utils import run_bass_kernel_spmd

    e = np.asarray(e, dtype=np.float32)
    h = np.asarray(h, dtype=np.float32)
    bo = np.asarray(bo, dtype=np.float32)
    n_chunks = e.shape[1]

    if n_chunks not in _prog_cache:
        _prog_cache[n_chunks] = _build(n_chunks)
    nc = _prog_cache[n_chunks]

    consts = _prep_consts(
        np.asarray(Wq, np.float32), np.asarray(bq, np.float32),
        np.asarray(Wk, np.float32), np.asarray(bk, np.float32),
        np.asarray(Wv, np.float32), np.asarray(bv, np.float32),
        np.asarray(Wo, np.float32), np.asarray(bo, np.float32),
        np.asarray(ln_g, np.float32), np.asarray(ln_b, np.float32))

    if np.any(bo):
        e = e + bo  # fold output bias into the residual

    in_maps = []
    for b in range(B):
        m = dict(consts)
        m["e"] = np.ascontiguousarray(e[b].reshape(n_chunks, R, D))
        m["h"] = np.ascontiguousarray(h[b])
        in_maps.append(m)

    res = run_bass_kernel_spmd(nc, in_maps, core_ids=list(range(B)))
    out = np.stack([r["out"] for r in res.results], axis=0)
    return out.reshape(B, n_chunks, N, S, D)
